# revision 21
# baseline (speedup 1.0000x reference)
"""Trainium2 Bass kernel for nn_K_attention_MH (sparse_attention).

Per token t (4096 total): X_t in R^{16x64} (heads x head_dim),
  D[i,j] = ||X_i - X_j||^2 ; K = exp(-sigma*D) ; Y = X + K @ X.

v2 strategy (pure data parallel over 8 cores, 512 tokens/core):
  - Host pre-permutes + casts x to fp16 in the exact SBUF layout
    [(t8,h16)=128 partitions, 64 groups, 64 d] so the input DMA is one
    contiguous stream (2KiB descriptors) and no on-chip cast is needed.
    Output y is stored fp16 in the same layout and inverse-permuted +
    cast back to fp32 on host. Device HBM traffic: 1 MiB in + 1 MiB out.
  - Per oct (8 groups of 8 tokens): PE transposes -> tts (X^T per group),
    sq = tts*tts on DVE (plain tensor_mul, 2x mode).
  - E built in PSUM: per group Gram (tts^T@tts) + row term (sq^T@(-0.5))
    + [col term & -BIG off-diagonal-block mask] merged in ONE matmul per
    s-half using stacked const|sq operand tiles:
      lhsT cmA = [-0.5 x64 ; masku x16],  rhs = sqA = [sq x64 ; maskv x16]
      lhsT cmB = [masku x16 ; -0.5 x64],  rhs = sqB = [maskv x16 ; sq x64]
  - W = exp(2*sigma*E) via one ACT op per oct; Y0 = W^T @ X per group on
    PE (W symmetric); y = Y0 + x on DVE; contiguous fp16 DMA out.
"""

import sys

for p in ("/opt/trn_rl_repo",):
    if p not in sys.path:
        sys.path.insert(0, p)

import numpy as np

import concourse.bass as bass
import concourse.mybir as mybir
import concourse.tile as tile
from concourse import bacc
from concourse.bass_utils import run_bass_kernel_spmd

N_CORES = 8
B, T, C = 2, 2048, 1024
H, HD = 16, 64
TOK = B * T                    # 4096 tokens total
TOK_PER_CORE = TOK // N_CORES  # 512
GROUPS = TOK_PER_CORE // 8     # 64 groups of 8 tokens
OCTS = GROUPS // 8             # 8 octs of 8 groups

F16 = mybir.dt.float16
F32 = mybir.dt.float32

MASK_S = 31616.0  # sqrt(~1e9), exactly representable in fp16


def _mask_consts():
    """Host-built fp16 mask constants for the merged col+mask matmuls."""
    S = np.float16(MASK_S)
    masku = np.zeros((16, 128), dtype=np.float16)
    for a in range(8):
        masku[a, 16 * a:16 * a + 16] = S
    maskv = np.zeros((16, 128), dtype=np.float16)
    for a in range(8):
        maskv[a, :] = -S
        maskv[a, 16 * a:16 * a + 16] = 0.0
    cm_a = np.full((80, 128), -0.5, dtype=np.float16)
    cm_a[64:80] = masku
    cm_b = np.zeros((128, 128), dtype=np.float16)
    cm_b[32:48] = masku
    cm_b[64:128] = -0.5
    sqv = np.broadcast_to(maskv[:, None, :], (16, 4, 128)).copy()
    sqbi = np.zeros((64, 4, 128), dtype=np.float16)
    sqbi[32:48] = sqv
    return cm_a, cm_b, sqv, sqbi


def build_kernel_body(ctx, nc, tc, x, sig, y, cm_a_d, cm_b_d, sqv_d, sqbi_d,
                      ident_d, prescale):
    consts = ctx.enter_context(tc.tile_pool(name="consts", bufs=1))
    sb = ctx.enter_context(tc.tile_pool(name="sb", bufs=3))
    ps_t = ctx.enter_context(tc.tile_pool(name="ps_t", bufs=2, space="PSUM"))
    ps_e = ctx.enter_context(tc.tile_pool(name="ps_e", bufs=2, space="PSUM"))
    ps_y = ctx.enter_context(tc.tile_pool(name="ps_y", bufs=2, space="PSUM"))

    # ---- one-time constants ----
    ident = consts.tile([128, 128], F16)
    if not prescale:
        # negh: -0.5 rows for the row-term matmul rhs (full 128 partitions
        # so slices share base partition with the per-s sq lhsT)
        negh = consts.tile([128, 128], F16)
        nc.vector.memset(negh, -0.5)

    # merged col+mask stationary operands, uploaded prebuilt from host
    # (lhsT/rhs must share base partition, and K>64 contractions must start
    # at partition 0, so the s=1 operand is zero-padded to K=128 with its
    # mask rows at the 32-aligned [32:48) range)
    cmA = consts.tile([80, 128], F16)   # rows 0-63 = -0.5, rows 64-79 = masku
    cmB = consts.tile([128, 128], F16)  # 32-47 masku, 64-127 -0.5, rest 0

    # static sq tiles (2 per parity for double buffering); mask/zero rows
    # uploaded once, sq rows rewritten each oct
    sqA = [consts.tile([80, 4, 128], F16, name=f"sqA{i}") for i in range(2)]
    sqB = [consts.tile([128, 4, 128], F16, name=f"sqB{i}") for i in range(2)]

    # broadcast 2*r_sigma to [128,1] fp32 for the ACT scale operand
    scale2s = consts.tile([128, 1], F32)
    nc.gpsimd.dma_start(out=scale2s, in_=sig[:].to_broadcast((128, 1)))
    nc.vector.tensor_scalar_mul(out=scale2s, in0=scale2s, scalar1=2.0)
    negsig = consts.tile([128, 1], F32)   # -sigma, for the prescale exp
    nc.vector.tensor_scalar_mul(out=negsig, in0=scale2s, scalar1=-0.5)

    # ---- persistent input tile; chunked contiguous loads on the sync
    # queue, with the const uploads interleaved after chunk 0 so oct 0 can
    # start while the rest streams in ----
    xt = consts.tile([128, GROUPS, HD], F16, tag="xt")
    nc.sync.dma_start(out=ident, in_=ident_d[:, :])
    nc.sync.dma_start(out=xt[:, 0:16, :], in_=x[:, 0:16, :])
    nc.sync.dma_start(out=cmA, in_=cm_a_d[:, :])
    nc.sync.dma_start(out=cmB, in_=cm_b_d[:, :])
    for i in range(2):
        nc.sync.dma_start(out=sqA[i][64:80, :, :], in_=sqv_d[:, :, :])
        nc.sync.dma_start(out=sqB[i][0:64, :, :], in_=sqbi_d[:, :, :])
    for c in range(1, 4):
        gs = slice(16 * c, 16 * (c + 1))
        nc.sync.dma_start(out=xt[:, gs, :], in_=x[:, gs, :])

    # ---- main loop, software-pipelined two-deep:
    #   body(o): T/copy/sq for oct o+1 (so DVE fills them during E(o)),
    #            then E(o), ACT(o), then Y/residual/store for oct o-1
    #            (so the PE works through the ACT(o) latency).
    def prep_oct(o):
        g0 = 8 * o
        pst = ps_t.tile([128, 4, 128], F16, tag="pst")
        for p in range(4):
            nc.tensor.transpose(
                pst[:, p, :], xt[:, g0 + 2 * p:g0 + 2 * p + 2, :], ident)
        tts = sb.tile([128, 4, 128], F16, tag="tts")
        nc.vector.tensor_copy(out=tts, in_=pst)
        sA, sB = sqA[o % 2], sqB[o % 2]
        nc.vector.tensor_mul(out=sA[0:64, :, :], in0=tts[0:64, :, :],
                             in1=tts[0:64, :, :])
        nc.vector.tensor_mul(out=sB[64:128, :, :], in0=tts[64:128, :, :],
                             in1=tts[64:128, :, :])
        if not prescale:
            return tts, sA, sB, xt[:, g0:g0 + 8, :]
        # prescale chain: xs = exp(-sigma*r2) * x replaces the row-term
        # matmuls (the col term in E supplies the output-row factor once
        # W is used as the Y stationary)
        xsl = xt[:, g0:g0 + 8, :]
        sq2 = sb.tile([128, 8, HD], F16, tag="sq2")
        nc.vector.tensor_mul(out=sq2, in0=xsl, in1=xsl)
        r2 = sb.tile([128, 8], F32, tag="r2")
        nc.vector.tensor_reduce(out=r2, in_=sq2, axis=mybir.AxisListType.X,
                                op=mybir.AluOpType.add)
        s16 = sb.tile([128, 8], F16, tag="s16")
        nc.scalar.activation(
            out=s16, in_=r2, func=mybir.ActivationFunctionType.Exp,
            bias=0.0, scale=negsig[:, 0:1],
        )
        xs = sb.tile([128, 8, HD], F16, tag="xs")
        s16b = bass.AP(tensor=s16.tensor, offset=s16.offset,
                       ap=[s16.ap[0], s16.ap[1], [0, HD]])
        nc.vector.tensor_mul(out=xs, in0=xsl, in1=s16b)
        return tts, sA, sB, xs

    def finish_oct(o, w, rhs):
        yb = ps_y.tile([128, 8, HD], F32, tag="yb")
        for p in range(4):
            for s in range(2):
                g = 2 * p + s
                nc.tensor.matmul(
                    yb[:, g, :], w[:, s, p, :], rhs[:, g, :],
                    start=True, stop=True, skip_group_check=True,
                )
        out = sb.tile([128, 8, HD], F16, tag="out")
        nc.vector.tensor_add(out=out, in0=yb, in1=xt[:, 8 * o:8 * o + 8, :])
        nc.sync.dma_start(out=y[:, 8 * o:8 * o + 8, :], in_=out)

    prev = None
    cur = prep_oct(0)
    for o in range(OCTS):
        tts, sA, sB, yrhs = cur
        if o + 1 < OCTS:
            cur = prep_oct(o + 1)

        # E in PSUM: per s-half bank: 4 Gram [+ 4 row] + 1 merged col/mask
        e = ps_e.tile([128, 2, 4, 128], F32, tag="e")
        w = sb.tile([128, 2, 4, 128], F16, tag="w")
        for s in range(2):
            ksl = slice(64 * s, 64 * (s + 1))
            for p in range(4):
                nc.tensor.matmul(
                    e[:, s, p, :], tts[ksl, p, :], tts[ksl, p, :],
                    start=(p == 0), stop=False, skip_group_check=True,
                )
            if not prescale:
                sq_t = sA if s == 0 else sB
                for p in range(4):
                    nc.tensor.matmul(
                        e[:, s, p, :], sq_t[ksl, p, :], negh[ksl, :],
                        start=False, stop=False, skip_group_check=True,
                    )
            if s == 0:
                nc.tensor.matmul(
                    e[:, 0, :, :], cmA, sA[0:80, :, :],
                    start=False, stop=True, skip_group_check=True,
                )
            else:
                nc.tensor.matmul(
                    e[:, 1, :, :], cmB, sB[:, :, :],
                    start=False, stop=True, skip_group_check=True,
                )
            # per-s ACT so exp(s=0) overlaps the s=1 matmuls
            nc.scalar.activation(
                out=w[:, s, :, :], in_=e[:, s, :, :],
                func=mybir.ActivationFunctionType.Exp,
                bias=0.0, scale=scale2s[:, 0:1],
            )

        if prev is not None:
            finish_oct(*prev)
        prev = (o, w, yrhs)

    finish_oct(*prev)


_NC_CACHE = {}


def build_nc(prescale=True):
    if prescale in _NC_CACHE:
        return _NC_CACHE[prescale]
    nc = bacc.Bacc("TRN2", target_bir_lowering=False, num_devices=N_CORES)
    x = nc.dram_tensor("x", [128, GROUPS, HD], F16, kind="ExternalInput")
    sig = nc.dram_tensor("r_sigma", [1], F32, kind="ExternalInput")
    cm_a_d = nc.dram_tensor("cm_a", [80, 128], F16, kind="ExternalInput")
    cm_b_d = nc.dram_tensor("cm_b", [128, 128], F16, kind="ExternalInput")
    sqv_d = nc.dram_tensor("sqv", [16, 4, 128], F16, kind="ExternalInput")
    sqbi_d = nc.dram_tensor("sqbi", [64, 4, 128], F16, kind="ExternalInput")
    ident_d = nc.dram_tensor("ident", [128, 128], F16, kind="ExternalInput")
    y = nc.dram_tensor("y", [128, GROUPS, HD], F16, kind="ExternalOutput")
    from contextlib import ExitStack
    with tile.TileContext(nc) as tc, ExitStack() as ctx:
        build_kernel_body(ctx, nc, tc, x, sig, y, cm_a_d, cm_b_d, sqv_d,
                          sqbi_d, ident_d, prescale)
    nc.compile()
    _NC_CACHE[prescale] = nc
    return nc


def make_in_maps(x: np.ndarray, r_sigma: np.ndarray):
    """Host-side shard + permute + cast to the device layout.

    Per core: tokens [512k, 512k+512) as (g64, t8, h16, d64) permuted to
    [(t,h)=128, g=64, d=64] fp16 contiguous."""
    xr = np.asarray(x, dtype=np.float32).reshape(N_CORES, GROUPS, 8, H, HD)
    xp = xr.transpose(0, 2, 3, 1, 4).reshape(N_CORES, 128, GROUPS, HD)
    x16 = np.ascontiguousarray(xp.astype(np.float16))
    sig = np.ascontiguousarray(np.asarray(r_sigma, dtype=np.float32))
    cm_a, cm_b, sqv, sqbi = _mask_consts()
    ident = np.eye(128, dtype=np.float16)
    return [{"x": x16[k], "r_sigma": sig, "cm_a": cm_a, "cm_b": cm_b,
             "sqv": sqv, "sqbi": sqbi, "ident": ident}
            for k in range(N_CORES)]


def unshard_output(results) -> np.ndarray:
    y16 = np.stack([r["y"] for r in results], axis=0)  # (8, 128, 64, 64)
    yr = y16.reshape(N_CORES, 8, H, GROUPS, HD).transpose(0, 3, 1, 2, 4)
    return np.ascontiguousarray(
        yr.astype(np.float32).reshape(B, T, C))


def kernel(x: np.ndarray, r_sigma: np.ndarray) -> np.ndarray:
    assert x.shape == (B, T, C) and x.dtype == np.float32
    # the prescale factorization computes exp(2*sigma*dot - sigma*r2) in
    # fp16, which can overflow for large sigma; fall back to the fully
    # masked symmetric variant (always-negative exponents) in that case
    nc = build_nc(prescale=float(np.asarray(r_sigma).ravel()[0]) <= 0.04)
    in_maps = make_in_maps(x, r_sigma)
    res = run_bass_kernel_spmd(nc, in_maps, core_ids=list(range(N_CORES)))
    return unshard_output(res.results)


if __name__ == "__main__":
    x = np.random.default_rng(0).standard_normal((B, T, C)).astype(np.float32)
    r_sigma = np.array([0.01], dtype=np.float32)
    y = kernel(x, r_sigma)
    print("ok", y.shape, y.dtype)


# revision 25
# speedup vs baseline: 1.1544x; 1.1544x over previous
"""Trainium2 Bass kernel for nn_K_attention_MH (sparse_attention).

Per token t (4096 total): X_t in R^{16x64} (heads x head_dim),
  D[i,j] = ||X_i - X_j||^2 ; K = exp(-sigma*D) ; Y = X + K @ X.

v2 strategy (pure data parallel over 8 cores, 512 tokens/core):
  - Host pre-permutes + casts x to fp16 in the exact SBUF layout
    [(t8,h16)=128 partitions, 64 groups, 64 d] so the input DMA is one
    contiguous stream (2KiB descriptors) and no on-chip cast is needed.
    Output y is stored fp16 in the same layout and inverse-permuted +
    cast back to fp32 on host. Device HBM traffic: 1 MiB in + 1 MiB out.
  - Per oct (8 groups of 8 tokens): PE transposes -> tts (X^T per group),
    sq = tts*tts on DVE (plain tensor_mul, 2x mode).
  - E built in PSUM: per group Gram (tts^T@tts) + row term (sq^T@(-0.5))
    + [col term & -BIG off-diagonal-block mask] merged in ONE matmul per
    s-half using stacked const|sq operand tiles:
      lhsT cmA = [-0.5 x64 ; masku x16],  rhs = sqA = [sq x64 ; maskv x16]
      lhsT cmB = [masku x16 ; -0.5 x64],  rhs = sqB = [maskv x16 ; sq x64]
  - W = exp(2*sigma*E) via one ACT op per oct; Y0 = W^T @ X per group on
    PE (W symmetric); y = Y0 + x on DVE; contiguous fp16 DMA out.
"""

import sys

for p in ("/opt/trn_rl_repo",):
    if p not in sys.path:
        sys.path.insert(0, p)

import numpy as np

import concourse.bass as bass
import concourse.mybir as mybir
import concourse.tile as tile
from concourse import bacc
from concourse.bass_utils import run_bass_kernel_spmd

N_CORES = 8
B, T, C = 2, 2048, 1024
H, HD = 16, 64
TOK = B * T                    # 4096 tokens total
TOK_PER_CORE = TOK // N_CORES  # 512
GROUPS = TOK_PER_CORE // 8     # 64 groups of 8 tokens
OCTS = GROUPS // 8             # 8 octs of 8 groups

F16 = mybir.dt.float16
F32 = mybir.dt.float32

MASK_S = 31616.0  # sqrt(~1e9), exactly representable in fp16


def _mask_consts():
    """Host-built fp16 mask constants for the merged col+mask matmuls."""
    S = np.float16(MASK_S)
    masku = np.zeros((16, 128), dtype=np.float16)
    for a in range(8):
        masku[a, 16 * a:16 * a + 16] = S
    maskv = np.zeros((16, 128), dtype=np.float16)
    for a in range(8):
        maskv[a, :] = -S
        maskv[a, 16 * a:16 * a + 16] = 0.0
    cm_a = np.full((80, 128), -0.5, dtype=np.float16)
    cm_a[64:80] = masku
    cm_b = np.zeros((128, 128), dtype=np.float16)
    cm_b[32:48] = masku
    cm_b[64:128] = -0.5
    sqv = np.broadcast_to(maskv[:, None, :], (16, 4, 128)).copy()
    sqbi = np.zeros((64, 4, 128), dtype=np.float16)
    sqbi[32:48] = sqv
    return cm_a, cm_b, sqv, sqbi


def build_kernel_body(ctx, nc, tc, x, sig, y, cm_a_d, cm_b_d, sqv_d, sqbi_d,
                      ident_d, prescale):
    consts = ctx.enter_context(tc.tile_pool(name="consts", bufs=1))
    sb = ctx.enter_context(tc.tile_pool(name="sb", bufs=3))
    ps_t = ctx.enter_context(tc.tile_pool(name="ps_t", bufs=2, space="PSUM"))
    ps_e = ctx.enter_context(tc.tile_pool(name="ps_e", bufs=2, space="PSUM"))
    ps_y = ctx.enter_context(tc.tile_pool(name="ps_y", bufs=2, space="PSUM"))

    # ---- one-time constants ----
    ident = consts.tile([128, 128], F16)
    if not prescale:
        # negh: -0.5 rows for the row-term matmul rhs (full 128 partitions
        # so slices share base partition with the per-s sq lhsT)
        negh = consts.tile([128, 128], F16)
        nc.vector.memset(negh, -0.5)

    # merged col+mask stationary operands, uploaded prebuilt from host
    # (lhsT/rhs must share base partition, and K>64 contractions must start
    # at partition 0, so the s=1 operand is zero-padded to K=128 with its
    # mask rows at the 32-aligned [32:48) range)
    cmA = consts.tile([80, 128], F16)   # rows 0-63 = -0.5, rows 64-79 = masku
    cmB = consts.tile([128, 128], F16)  # 32-47 masku, 64-127 -0.5, rest 0

    # static sq tiles (2 per parity for double buffering); mask/zero rows
    # uploaded once, sq rows rewritten each oct
    sqA = [consts.tile([80, 4, 128], F16, name=f"sqA{i}") for i in range(2)]
    sqB = [consts.tile([128, 4, 128], F16, name=f"sqB{i}") for i in range(2)]

    # broadcast 2*r_sigma to [128,1] fp32 for the ACT scale operand
    scale2s = consts.tile([128, 1], F32)
    nc.gpsimd.dma_start(out=scale2s, in_=sig[:].to_broadcast((128, 1)))
    nc.vector.tensor_scalar_mul(out=scale2s, in0=scale2s, scalar1=2.0)
    negsig = consts.tile([128, 1], F32)   # -sigma, for the prescale exp
    nc.vector.tensor_scalar_mul(out=negsig, in0=scale2s, scalar1=-0.5)

    # ---- persistent input tile; chunked contiguous loads on the sync
    # queue, with the const uploads interleaved after chunk 0 so oct 0 can
    # start while the rest streams in ----
    xt = consts.tile([128, GROUPS, HD], F16, tag="xt")
    nc.sync.dma_start(out=ident, in_=ident_d[:, :])
    nc.sync.dma_start(out=xt[:, 0:16, :], in_=x[:, 0:16, :])
    nc.sync.dma_start(out=cmA, in_=cm_a_d[:, :])
    nc.sync.dma_start(out=cmB, in_=cm_b_d[:, :])
    for i in range(2):
        nc.sync.dma_start(out=sqA[i][64:80, :, :], in_=sqv_d[:, :, :])
        nc.sync.dma_start(out=sqB[i][0:64, :, :], in_=sqbi_d[:, :, :])
    for c in range(1, 4):
        gs = slice(16 * c, 16 * (c + 1))
        nc.sync.dma_start(out=xt[:, gs, :], in_=x[:, gs, :])

    # ---- main loop, software-pipelined two-deep:
    #   body(o): T/copy/sq for oct o+1 (so DVE fills them during E(o)),
    #            then E(o), ACT(o), then Y/residual/store for oct o-1
    #            (so the PE works through the ACT(o) latency).
    def prep_oct(o):
        g0 = 8 * o
        pst = ps_t.tile([128, 4, 128], F16, tag="pst")
        for p in range(4):
            nc.tensor.transpose(
                pst[:, p, :], xt[:, g0 + 2 * p:g0 + 2 * p + 2, :], ident)
        tts = sb.tile([128, 4, 128], F16, tag="tts")
        nc.vector.tensor_copy(out=tts, in_=pst)
        sA, sB = sqA[o % 2], sqB[o % 2]
        nc.vector.tensor_mul(out=sA[0:64, :, :], in0=tts[0:64, :, :],
                             in1=tts[0:64, :, :])
        nc.vector.tensor_mul(out=sB[64:128, :, :], in0=tts[64:128, :, :],
                             in1=tts[64:128, :, :])
        return tts, sA, sB

    def prep_xs(o):
        # prescale chain: xs = exp(-sigma*r2) * x replaces the row-term
        # matmuls (the col term in E supplies the output-row factor once
        # W is used as the Y stationary). Issued after E(o-1) so it sits
        # behind the W ACTs in the scalar queue and never delays them.
        xsl = xt[:, 8 * o:8 * o + 8, :]
        sq2 = sb.tile([128, 8, HD], F16, tag="sq2")
        nc.vector.tensor_mul(out=sq2, in0=xsl, in1=xsl)
        r2 = sb.tile([128, 8], F32, tag="r2")
        nc.vector.tensor_reduce(out=r2, in_=sq2, axis=mybir.AxisListType.X,
                                op=mybir.AluOpType.add)
        s16 = sb.tile([128, 8], F16, tag="s16")
        nc.scalar.activation(
            out=s16, in_=r2, func=mybir.ActivationFunctionType.Exp,
            bias=0.0, scale=negsig[:, 0:1],
        )
        xs = sb.tile([128, 8, HD], F16, tag="xs")
        s16b = bass.AP(tensor=s16.tensor, offset=s16.offset,
                       ap=[s16.ap[0], s16.ap[1], [0, HD]])
        nc.vector.tensor_mul(out=xs, in0=xsl, in1=s16b)
        return xs

    def finish_oct(o, w, rhs):
        yb = ps_y.tile([128, 8, HD], F32, tag="yb")
        for p in range(4):
            for s in range(2):
                g = 2 * p + s
                nc.tensor.matmul(
                    yb[:, g, :], w[:, s, p, :], rhs[:, g, :],
                    start=True, stop=True, skip_group_check=True,
                )
        out = sb.tile([128, 8, HD], F16, tag="out")
        nc.vector.tensor_add(out=out, in0=yb, in1=xt[:, 8 * o:8 * o + 8, :])
        nc.sync.dma_start(out=y[:, 8 * o:8 * o + 8, :], in_=out)

    prev = None
    cur = prep_oct(0)
    cur_xs = prep_xs(0) if prescale else xt[:, 0:8, :]
    for o in range(OCTS):
        tts, sA, sB = cur
        yrhs = cur_xs
        if o + 1 < OCTS:
            cur = prep_oct(o + 1)

        # E in PSUM: per s-half bank: 4 Gram [+ 4 row] + 1 merged col/mask.
        # Separate tiles per s-half so the s=0 exp (ACT) doesn't create a
        # false tile-level dependency against the s=1 matmul writes.
        w = sb.tile([128, 2, 4, 128], F16, tag="w")
        for s in range(2):
            e = ps_e.tile([128, 4, 128], F32, tag=f"e{s}")
            ksl = slice(64 * s, 64 * (s + 1))
            for p in range(4):
                nc.tensor.matmul(
                    e[:, p, :], tts[ksl, p, :], tts[ksl, p, :],
                    start=(p == 0), stop=False, skip_group_check=True,
                )
            if not prescale:
                sq_t = sA if s == 0 else sB
                for p in range(4):
                    nc.tensor.matmul(
                        e[:, p, :], sq_t[ksl, p, :], negh[ksl, :],
                        start=False, stop=False, skip_group_check=True,
                    )
            if s == 0:
                nc.tensor.matmul(
                    e[:, :, :], cmA, sA[0:80, :, :],
                    start=False, stop=True, skip_group_check=True,
                )
            else:
                nc.tensor.matmul(
                    e[:, :, :], cmB, sB[:, :, :],
                    start=False, stop=True, skip_group_check=True,
                )
            # per-s ACT so exp(s=0) overlaps the s=1 matmuls
            nc.scalar.activation(
                out=w[:, s, :, :], in_=e,
                func=mybir.ActivationFunctionType.Exp,
                bias=0.0, scale=scale2s[:, 0:1],
            )

        if o + 1 < OCTS:
            cur_xs = (prep_xs(o + 1) if prescale
                      else xt[:, 8 * (o + 1):8 * (o + 1) + 8, :])
        if prev is not None:
            finish_oct(*prev)
        prev = (o, w, yrhs)

    finish_oct(*prev)


_NC_CACHE = {}


def build_nc(prescale=True):
    if prescale in _NC_CACHE:
        return _NC_CACHE[prescale]
    nc = bacc.Bacc("TRN2", target_bir_lowering=False, num_devices=N_CORES)
    x = nc.dram_tensor("x", [128, GROUPS, HD], F16, kind="ExternalInput")
    sig = nc.dram_tensor("r_sigma", [1], F32, kind="ExternalInput")
    cm_a_d = nc.dram_tensor("cm_a", [80, 128], F16, kind="ExternalInput")
    cm_b_d = nc.dram_tensor("cm_b", [128, 128], F16, kind="ExternalInput")
    sqv_d = nc.dram_tensor("sqv", [16, 4, 128], F16, kind="ExternalInput")
    sqbi_d = nc.dram_tensor("sqbi", [64, 4, 128], F16, kind="ExternalInput")
    ident_d = nc.dram_tensor("ident", [128, 128], F16, kind="ExternalInput")
    y = nc.dram_tensor("y", [128, GROUPS, HD], F16, kind="ExternalOutput")
    from contextlib import ExitStack
    with tile.TileContext(nc) as tc, ExitStack() as ctx:
        build_kernel_body(ctx, nc, tc, x, sig, y, cm_a_d, cm_b_d, sqv_d,
                          sqbi_d, ident_d, prescale)
    nc.compile()
    _NC_CACHE[prescale] = nc
    return nc


def make_in_maps(x: np.ndarray, r_sigma: np.ndarray):
    """Host-side shard + permute + cast to the device layout.

    Per core: tokens [512k, 512k+512) as (g64, t8, h16, d64) permuted to
    [(t,h)=128, g=64, d=64] fp16 contiguous."""
    xr = np.asarray(x, dtype=np.float32).reshape(N_CORES, GROUPS, 8, H, HD)
    xp = xr.transpose(0, 2, 3, 1, 4).reshape(N_CORES, 128, GROUPS, HD)
    x16 = np.ascontiguousarray(xp.astype(np.float16))
    sig = np.ascontiguousarray(np.asarray(r_sigma, dtype=np.float32))
    cm_a, cm_b, sqv, sqbi = _mask_consts()
    ident = np.eye(128, dtype=np.float16)
    return [{"x": x16[k], "r_sigma": sig, "cm_a": cm_a, "cm_b": cm_b,
             "sqv": sqv, "sqbi": sqbi, "ident": ident}
            for k in range(N_CORES)]


def unshard_output(results) -> np.ndarray:
    y16 = np.stack([r["y"] for r in results], axis=0)  # (8, 128, 64, 64)
    yr = y16.reshape(N_CORES, 8, H, GROUPS, HD).transpose(0, 3, 1, 2, 4)
    return np.ascontiguousarray(
        yr.astype(np.float32).reshape(B, T, C))


def kernel(x: np.ndarray, r_sigma: np.ndarray) -> np.ndarray:
    assert x.shape == (B, T, C) and x.dtype == np.float32
    # the prescale factorization computes exp(2*sigma*dot - sigma*r2) in
    # fp16, which can overflow for large sigma; fall back to the fully
    # masked symmetric variant (always-negative exponents) in that case
    nc = build_nc(prescale=float(np.asarray(r_sigma).ravel()[0]) <= 0.04)
    in_maps = make_in_maps(x, r_sigma)
    res = run_bass_kernel_spmd(nc, in_maps, core_ids=list(range(N_CORES)))
    return unshard_output(res.results)


if __name__ == "__main__":
    x = np.random.default_rng(0).standard_normal((B, T, C)).astype(np.float32)
    r_sigma = np.array([0.01], dtype=np.float32)
    y = kernel(x, r_sigma)
    print("ok", y.shape, y.dtype)
